# revision 3
# baseline (speedup 1.0000x reference)
"""IntLoRA-SHIFT fused kernel for Trainium2 (8 NeuronCores, tensor-parallel on out_features).

Math (per reference):
    w_int  = ori_weight_round - zero_point                    [O, I]
    lora   = (aux_R + loraB @ loraA) / where(w_int==0, 1, w_int)
    wu     = delta + lora
    weight = sign(wu) * 2^round(log2|wu|) * w_int
    out    = x @ weight.T + bias

v2 design (per core, O sharded 8 x 512):
  The baseline ran weight-prep (Phase A, ~150 us of DVE work) strictly before
  the matmul stream (Phase B, ~450 us of PE work).  v2 software-pipelines the
  two phases:

  * Prep runs in TRANSPOSED layout: the host pre-transposes w_int = ori - zp
    (bf16: exact, |w_int| <= 255) and aux (fp32) to [i, osh] so each 128-row
    k-slice of the bf16 weight wT[i,k,o] is produced directly by a short DVE
    chain -- no on-device transpose at all.  Per-out-channel delta/bias become
    [128, osh] broadcast tiles built once via K=1 matmuls.
  * Phase B runs in 5 passes over k-groups (3,4,6,8,11 slices of 128).
    Pass partials accumulate in a bf16 SBUF accumulator acc[128, 64, 512]:
    ScalarE drains each PSUM tile to a bf16 tmp tile; DVE adds 4 token-tiles
    at a time (2048-wide ops) into acc.  Pass 0 adds the bias broadcast; the
    last pass emits fp32 and DMAs out.  Prep of group j+2 interleaves into
    pass j's emission (acc chunks first, then prep, then matmuls) so DVE prep
    hides under PE matmuls; only group 0's prep (~20 us) is exposed.
  * PE work is just the 2048 main bf16 matmuls (N=512, ~216 ns each) plus 32
    tiny rank-4 (loraB@loraA) matmuls: the critical path (sim: PE busy 467 us
    of 489 total; DVE 190, ACT 200, Pool 67, DMA 268).
  * Final-pass accumulator adds run on GPSIMD (Pool) so DVE is idle during
    the last pass; in a sustained For_i loop the next iteration's weight prep
    overlaps it.  Host-side input prep (layout/constant folding): x -> bf16
    k-major transpose; w_int = ori - zp shipped as exact bf16; delta folded
    into aux as aux + delta*w_int (identical wu wherever w_int != 0, and the
    weight is q*w_int = 0 where w_int == 0 either way); loraB pre-transposed.
    Measured: rel err 9.56e-3; ~560-615 us/iteration by For_i slope on 8 axon
    NCs (vs 609-660 us for the previous phase-serial version).
"""
import os
import sys

for _p in ("/root/.axon_site", "/root/.axon_site/_ro/trn_rl_repo", "/root/.axon_site/_ro/pypackages", "/opt/trn_rl_repo"):
    if os.path.isdir(_p) and _p not in sys.path:
        sys.path.append(_p)

import numpy as np

import concourse.bacc as bacc
import concourse.mybir as mybir
import concourse.tile as tile
from concourse.bass_utils import run_bass_kernel_spmd

A = mybir.AluOpType
F32 = mybir.dt.float32
BF16 = mybir.dt.bfloat16
I32 = mybir.dt.int32

C_ROUND = 0x004AFB0C                                   # carry threshold: mantissa >= sqrt(2)
EXP_MASK = int(np.uint32(0xFF800000).view(np.int32))   # sign+exponent mask

# full problem config
FULL = dict(tok=8192, i=4096, osh=512, r=4, n_cores=8)
B_, S_, O_ = 4, 2048, 4096

# k-slice pass groups: pass j consumes wT slices [ks, ke); group j+2 preps
# during pass j (groups 0 and 1 are emitted up front).
GROUPS = [(0, 3), (3, 7), (7, 13), (13, 21), (21, 32)]


def build(tok, i, osh, r, n_cores, reps=1):
    """Build + compile the per-core kernel (SPMD: same program, sharded data).

    reps>1 wraps the whole body in a hardware For_i loop (for benchmarking:
    one dispatch executes the body `reps` times)."""
    nk = i // 128          # contraction k-slices
    ntt = tok // 128       # token tiles
    ntg = tok // 512       # token groups (4 tiles each)
    assert GROUPS[-1][1] == nk

    nc = bacc.Bacc("TRN2", target_bir_lowering=False, debug=False,
                   enable_asserts=False, num_devices=n_cores)
    xt_d = nc.dram_tensor("xtr", [nk, 128, tok], BF16, kind="ExternalInput").ap()
    wintT_d = nc.dram_tensor("wintT", [i, osh], BF16, kind="ExternalInput").ap()
    auxT_d = nc.dram_tensor("auxT", [i, osh], F32, kind="ExternalInput").ap()
    bsr_d = nc.dram_tensor("bsr", [1, osh], F32, kind="ExternalInput").ap()
    la_d = nc.dram_tensor("la", [r, i], F32, kind="ExternalInput").ap()      # loraA
    bt_d = nc.dram_tensor("bt", [r, osh], F32, kind="ExternalInput").ap()    # loraB shard, pre-transposed
    out_d = nc.dram_tensor("out", [tok, osh], F32, kind="ExternalOutput").ap()

    import contextlib

    with tile.TileContext(nc) as tc:
        with tc.tile_pool(name="const", bufs=1) as cp, \
             tc.tile_pool(name="prep", bufs=2) as pr, \
             tc.tile_pool(name="bpool", bufs=2) as bp, \
             tc.tile_pool(name="dpool", bufs=2) as dp, \
             tc.tile_pool(name="pba", bufs=2, space="PSUM") as pba, \
             tc.tile_pool(name="pps", bufs=6, space="PSUM") as pps, \
             (tc.For_i(0, reps, 1) if reps > 1 else contextlib.nullcontext()):

            # ---- constants
            bt_sb = cp.tile([r, osh], F32)
            nc.sync.dma_start(bt_sb[:], bt_d[:])
            ones_sb = cp.tile([1, 128], F32)
            nc.vector.memset(ones_sb[:], 1.0)

            # per-out-channel params as [128, osh] broadcasts (K=1 matmuls)
            def bcast(src_d, dtype, copies=1):
                row = cp.tile([1, osh], F32, tag=f"row{src_d.tensor.name}")
                nc.sync.dma_start(row[:], src_d[:])
                ps = pba.tile([128, osh], F32, tag="ba")
                nc.tensor.matmul(ps[:], ones_sb[:, :], row[:, :], start=True, stop=True)
                t = cp.tile([128, copies, osh], dtype, tag=f"bc{src_d.tensor.name}")
                for m in range(copies):
                    nc.scalar.copy(t[:, m, :], ps[:])
                return t

            bias_b4 = bcast(bsr_d, BF16, copies=4)

            # resident transposed weight [i(128), k, o] and bf16 accumulator
            wT = cp.tile([128, nk, osh], BF16)
            acc = cp.tile([128, ntt, osh], BF16)

            # ---- weight-prep machinery (one k-slice = 128 i-rows x osh)
            dma_tiles = {}
            next_dma = [0]

            def prep_dma(s):
                o = pr.tile([128, osh], BF16, tag="wint", bufs=4)
                a = pr.tile([128, osh], F32, tag="aux", bufs=3)
                l = pr.tile([r, 128], F32, tag="lac", bufs=3)
                nc.sync.dma_start(o[:], wintT_d[s * 128:(s + 1) * 128, :])
                nc.sync.dma_start(a[:], auxT_d[s * 128:(s + 1) * 128, :])
                nc.sync.dma_start(l[:], la_d[:, s * 128:(s + 1) * 128])
                dma_tiles[s] = (o, a, l)

            def issue_dmas(upto):
                while next_dma[0] < min(upto, nk):
                    prep_dma(next_dma[0])
                    next_dma[0] += 1

            def prep_chain(s):
                issue_dmas(s + 3)
                o, a, l = dma_tiles.pop(s)
                ps_ba = pba.tile([128, osh], F32, tag="ba")
                nc.tensor.matmul(ps_ba[:], l[:], bt_sb[:], start=True, stop=True)
                den = pr.tile([128, osh], F32, tag="den")
                nc.vector.scalar_tensor_tensor(den[:], o[:], 0.0, o[:],
                                               A.is_equal, A.add)
                rcp = pr.tile([128, osh], F32, tag="rcp")
                nc.vector.reciprocal_approx_fast(rcp[:], den[:])
                nc.vector.tensor_tensor(a[:], a[:], ps_ba[:], A.add)      # num
                nc.vector.tensor_tensor(den[:], a[:], rcp[:], A.mult)     # wu (den dead)
                nc.vector.tensor_scalar(rcp[:].bitcast(I32), den[:].bitcast(I32),
                                        C_ROUND, None, A.add)
                nc.vector.tensor_scalar(rcp[:].bitcast(I32), rcp[:].bitcast(I32),
                                        EXP_MASK, None, A.bitwise_and)
                nc.vector.tensor_tensor(wT[:, s, :], rcp[:], o[:], A.mult)

            # groups 0+1 up front (pass 0 only waits on group 0; DVE keeps
            # running group 1 under pass 0's matmuls)
            issue_dmas(3)
            for s in range(GROUPS[1][1]):
                prep_chain(s)
            prep_sched = {j - 2: list(range(*GROUPS[j])) for j in range(2, len(GROUPS))}

            # ---- Phase B passes
            n_pass = len(GROUPS)
            for pj, (ks, ke) in enumerate(GROUPS):
                sg = ke - ks
                todo = prep_sched.get(pj, [])
                per_tg = [todo[(t * len(todo)) // ntg:((t + 1) * len(todo)) // ntg]
                          for t in range(ntg)]
                backlog = []

                def emit_acc(tg, tmp4):
                    asl = acc[:, tg * 4:(tg + 1) * 4, :]
                    if pj == 0:
                        nc.vector.tensor_tensor(asl, tmp4[:], bias_b4[:], A.add)
                    elif pj < n_pass - 1:
                        nc.vector.tensor_tensor(asl, asl, tmp4[:], A.add)
                    else:
                        os4 = bp.tile([128, 4, osh], F32, tag="os", bufs=2)
                        nc.gpsimd.tensor_tensor(os4[:], asl, tmp4[:], A.add)
                        nc.scalar.dma_start(
                            out_d[tg * 512:(tg + 1) * 512, :]
                            .rearrange("(a p) o -> p a o", p=128), os4[:])

                for tg in range(ntg):
                    while len(backlog) > 3:
                        emit_acc(*backlog.pop(0))
                    for s in per_tg[tg]:
                        prep_chain(s)
                    xt = bp.tile([128, 16, 512], BF16, tag="xt", bufs=2)
                    nc.sync.dma_start(
                        xt[:, :sg, :],
                        xt_d[ks:ke, :, tg * 512:(tg + 1) * 512].rearrange("k p t -> p k t"))
                    tmp4 = dp.tile([128, 4, osh], BF16, tag="tmp", bufs=6)
                    for ts in range(4):
                        ps = pps.tile([128, osh], F32, tag="ps")
                        for j2 in range(sg):
                            nc.tensor.matmul(ps[:], xt[:, j2, ts * 128:(ts + 1) * 128],
                                             wT[:, ks + j2, :],
                                             start=(j2 == 0), stop=(j2 == sg - 1))
                        nc.scalar.copy(tmp4[:, ts, :], ps[:])
                    backlog.append((tg, tmp4))
                while backlog:
                    emit_acc(*backlog.pop(0))

    nc.compile()
    return nc


_CACHE = {}


def _get(cfg_key):
    if cfg_key not in _CACHE:
        _CACHE[cfg_key] = build(**dict(cfg_key))
    return _CACHE[cfg_key]


def make_in_maps(x2d, ori, delta, zp, aux, laA, laB, bias, n_cores, osh):
    import ml_dtypes
    nk = x2d.shape[1] // 128
    xtr = np.ascontiguousarray(x2d.astype(ml_dtypes.bfloat16).T).reshape(nk, 128, x2d.shape[0])
    wint = ori - zp.reshape(-1, 1)          # exact integers in [-255, 255]
    # fold the per-row delta into aux: (aux + dl*wint + BA)/wint_safe ==
    # dl + (aux + BA)/wint_safe wherever wint != 0; where wint == 0 the final
    # weight is q*wint = 0 either way, so wu may differ freely there.
    auxf = aux + delta.reshape(-1, 1) * wint
    in_maps = []
    for c in range(n_cores):
        sl = slice(c * osh, (c + 1) * osh)
        in_maps.append({
            "xtr": xtr,
            "wintT": np.ascontiguousarray(wint[sl].T.astype(ml_dtypes.bfloat16)),
            "auxT": np.ascontiguousarray(auxf[sl].T),
            "bsr": np.ascontiguousarray(bias[sl].reshape(-1)[None, :]),
            "la": laA,
            "bt": np.ascontiguousarray(laB[sl].T),
        })
    return in_maps


def kernel(x, ori_weight_round, weight_quant_delta, weight_quant_zero_point,
           aux_R, loraA_w, loraB_w, bias, _trace=False):
    cfg = FULL
    n_cores, osh = cfg["n_cores"], cfg["osh"]
    x2d = np.ascontiguousarray(np.asarray(x, dtype=np.float32).reshape(cfg["tok"], cfg["i"]))
    nc = _get(tuple(sorted(cfg.items())))
    in_maps = make_in_maps(
        x2d,
        np.asarray(ori_weight_round, np.float32),
        np.asarray(weight_quant_delta, np.float32),
        np.asarray(weight_quant_zero_point, np.float32),
        np.asarray(aux_R, np.float32),
        np.asarray(loraA_w, np.float32),
        np.asarray(loraB_w, np.float32),
        np.asarray(bias, np.float32),
        n_cores, osh)
    res = run_bass_kernel_spmd(nc, in_maps, core_ids=list(range(n_cores)), trace=_trace)
    out = np.concatenate([res.results[c]["out"] for c in range(n_cores)], axis=1)
    out = out.reshape(B_, S_, O_)
    if _trace:
        return out, res
    return out
